# revision 1
# baseline (speedup 1.0000x reference)
"""Trainium2 Bass kernel for nn_BasicRecurrentEntityEncoder.

Data-parallel over batch B=256 across 8 NeuronCores (32 batches/core).
Per core, entity rows are laid out k-major: j = k*32 + b (K padded 30->32),
giving 1024 rows = 8 chunks x 128 partitions with b = p % 32 uniform in
every chunk. State is kept both natural ([128, 8, 256], row-major) and
transposed ([d, j], for PE matmuls); the transpose is refreshed on PE each
step via identity matmuls.

Phase A: indirect-DMA gather of bf16 embedding rows (masked tokens point at
a zero row), on-chip bag-of-words sums, encT / e@W / e.keys (EK) / keys@V
precomputes. Phase B: the 64-step recurrence with PE doing h@U, the gate
dot h.e, and the kV/eW additions (identity/selection matmuls into PSUM).
"""

import os
import numpy as np
import ml_dtypes

B, S, L, D, K, VOCAB = 256, 64, 16, 256, 30, 50000
NCORES = 8
BL = B // NCORES          # 32 batches per core
KH = 32                   # padded K
J = KH * BL               # 1024 rows per core
CH = 8                    # row chunks (128 partitions each)
GRP = 8                   # gather groups
SPG = S // GRP            # steps per group
VPAD = VOCAB + 8          # embedding rows incl. zero pad rows
NEG = -60.0               # gate logit offset for masked sentences
EPS = 1e-12

LAST_EXEC_NS = None       # set when BASS_KERNEL_TRACE=1
NSTEPS = int(os.environ.get("BK_NSTEPS", str(S)))
SKIP_GATHER = os.environ.get("BK_SKIP_GATHER", "0") == "1"

_bf16 = ml_dtypes.bfloat16


def _build_nc():
    import concourse.bacc as bacc
    import concourse.bass as bass
    import concourse.mybir as mybir
    from concourse import tile

    f32 = mybir.dt.float32
    bf16 = mybir.dt.bfloat16
    i32 = mybir.dt.int32
    MULT = mybir.AluOpType.mult
    ADD = mybir.AluOpType.add
    MAX = mybir.AluOpType.max

    nc = bacc.Bacc("TRN2", target_bir_lowering=False, debug=False,
                   num_devices=NCORES)

    # ---- DRAM parameters -------------------------------------------------
    emb = nc.dram_tensor("emb", [8192, 4 * D], bf16, kind="ExternalInput")
    idx_d = nc.dram_tensor("idx", [128, 512], mybir.dt.int16, kind="ExternalInput")
    keysT_d = nc.dram_tensor("keysT", [128, 2, J], bf16, kind="ExternalInput")
    u_d = nc.dram_tensor("u", [128, 2, D], bf16, kind="ExternalInput")
    v_d = nc.dram_tensor("v", [128, 2, D], bf16, kind="ExternalInput")
    w_d = nc.dram_tensor("w", [128, 2, D], bf16, kind="ExternalInput")
    bias_d = nc.dram_tensor("bias", [128, S], f32, kind="ExternalInput")
    selsum_d = nc.dram_tensor("selsum", [128, BL], bf16, kind="ExternalInput")
    selkm_d = nc.dram_tensor("selkm", [BL, 128], bf16, kind="ExternalInput")
    mdiag_d = nc.dram_tensor("mdiag", [128, BL], f32, kind="ExternalInput")
    ident_d = nc.dram_tensor("ident", [128, 128], bf16, kind="ExternalInput")
    y_d = nc.dram_tensor("y", [BL, K, D], f32, kind="ExternalOutput")
    dbg_d = nc.dram_tensor("dbg", [128, 2], f32, kind="ExternalOutput")

    with tile.TileContext(nc) as tc:
        ctxs = []

        def pool(name, bufs, space="SBUF"):
            p = tc.tile_pool(name=name, bufs=bufs, space=space)
            ctxs.append(p)
            return p.__enter__()

        persist = pool("persist", 1)
        gbuf = pool("gbuf", 2)
        scratch = pool("scratch", 1)
        ps_big = pool("ps_big", 1, "PSUM")      # [128, 2048] = 4 banks
        ps_sm = pool("ps_sm", 2, "PSUM")        # [128, 256] slots, shared tag
        ps_t = pool("ps_t", 1, "PSUM")          # [128, 1024] = 2 banks

        # ---- persistent SBUF tensors ------------------------------------
        idx_sb = persist.tile([128, 512], mybir.dt.int16, tag="idx")
        keysT = persist.tile([128, 2, J], bf16, tag="keysT")
        u_sb = persist.tile([128, 2, D], bf16, tag="u")
        v_sb = persist.tile([128, 2, D], bf16, tag="v")
        w_sb = persist.tile([128, 2, D], bf16, tag="w")
        bias_sb = persist.tile([128, S], f32, tag="bias")
        selsum = persist.tile([128, BL], bf16, tag="selsum")
        selkm = persist.tile([BL, 128], bf16, tag="selkm")
        mdiag = persist.tile([128, BL], f32, tag="mdiag")
        ident = persist.tile([128, 128], bf16, tag="ident")
        encT = persist.tile([128, 2, S * BL], bf16, tag="encT")
        ew_all = persist.tile([BL, S * D], bf16, tag="ew")
        ekm = persist.tile([128, CH, S], f32, tag="ekm")
        kv = persist.tile([128, CH, D], bf16, tag="kv")
        h_nat = persist.tile([128, CH, D], bf16, tag="h_nat")
        hT = persist.tile([128, 2, J], bf16, tag="hT")
        h_tld = persist.tile([128, CH, D], bf16, tag="h_tld")
        upd = persist.tile([128, CH, D], bf16, tag="upd")
        sqd = persist.tile([128, CH, D], bf16, tag="sqd")
        gm = persist.tile([128, CH, BL], f32, tag="gm")
        gpre = persist.tile([128, CH], f32, tag="gpre")
        gp2 = persist.tile([128, CH], f32, tag="gp2")
        g_all = persist.tile([128, CH], f32, tag="g_all")
        ss = persist.tile([128, CH], f32, tag="ss")
        ssq = persist.tile([128, CH], f32, tag="ssq")
        r_all = persist.tile([128, CH], f32, tag="r_all")
        hf32 = persist.tile([128, CH, D], f32, tag="hf32")
        epsv = persist.tile([128, 1], f32, tag="epsv")
        dum1 = persist.tile([128, 1], f32, tag="dum1")
        dum2 = persist.tile([128, 1], f32, tag="dum2")

        # ---- load parameters --------------------------------------------
        nc.sync.dma_start(out=idx_sb[:], in_=idx_d.ap())
        nc.sync.dma_start(out=keysT[:], in_=keysT_d.ap())
        nc.sync.dma_start(out=u_sb[:], in_=u_d.ap())
        nc.sync.dma_start(out=v_sb[:], in_=v_d.ap())
        nc.sync.dma_start(out=w_sb[:], in_=w_d.ap())
        nc.sync.dma_start(out=bias_sb[:], in_=bias_d.ap())
        nc.sync.dma_start(out=selsum[:], in_=selsum_d.ap())
        nc.sync.dma_start(out=selkm[:], in_=selkm_d.ap())
        nc.sync.dma_start(out=mdiag[:], in_=mdiag_d.ap())
        nc.sync.dma_start(out=ident[:], in_=ident_d.ap())

        nc.vector.memset(epsv[:], EPS)
        nc.vector.memset(dum1[:], 1.0)
        nc.vector.memset(h_nat[:], 0.0)
        nc.vector.memset(hT[:], 0.0)

        # ========== interleaved: gathers + per-group precompute + scan ====
        def emit_gather(g):
            raw = gbuf.tile([128, 4 * SPG, D], bf16, tag="raw")
            nc.gpsimd.dma_gather(
                out_ap=raw[:].rearrange("p (q k) d -> p q (k d)", k=4),
                in_ap=emb.ap(),
                idxs_ap=idx_sb[:, g * 64:(g + 1) * 64],
                num_idxs=1024, num_idxs_reg=1024, elem_size=4 * D)
            return raw

        def emit_group_precompute(g, raw):
            # l-sum: raw[p, (s_in, l_hi), d] -> part[p, s_in, d]
            s02 = gbuf.tile([128, SPG, 2, D], bf16, tag="s02")
            r4 = raw[:].rearrange("p (s l) d -> p s l d", l=4)
            nc.vector.tensor_tensor(out=s02[:], in0=r4[:, :, 0:2, :],
                                    in1=r4[:, :, 2:4, :], op=ADD)
            part = gbuf.tile([128, SPG, D], bf16, tag="part")
            nc.vector.tensor_tensor(out=part[:], in0=s02[:, :, 0, :],
                                    in1=s02[:, :, 1, :], op=ADD)
            # encT[half][d, (s, b)] via PE: part.T @ selsum
            for half in range(2):
                etp = ps_sm.tile([128, SPG * BL], f32, tag="sm")
                for si in range(SPG):
                    nc.tensor.matmul(
                        out=etp[:, si * BL:(si + 1) * BL],
                        lhsT=part[:, si, half * 128:(half + 1) * 128],
                        rhs=selsum[:], start=(si == 0), stop=(si == SPG - 1))
                nc.vector.tensor_copy(
                    out=encT[:, half, g * SPG * BL:(g + 1) * SPG * BL],
                    in_=etp[:])
            # eW[b, (s, d)] for this group
            ewp = ps_big.tile([BL, 8 * D], f32, tag="big")
            for si in range(SPG):
                s = g * SPG + si
                for half in range(2):
                    nc.tensor.matmul(
                        out=ewp[:, si * D:(si + 1) * D],
                        lhsT=encT[:, half, s * BL:(s + 1) * BL],
                        rhs=w_sb[:, half, :],
                        start=(half == 0 and si % 2 == 0),
                        stop=(half == 1 and si % 2 == 1))
            nc.vector.tensor_copy(out=ew_all[:, g * 8 * D:(g + 1) * 8 * D],
                                  in_=ewp[:])
            # EK for this group -> ekm[:, :, 8g:8g+8]
            gbig = ps_big.tile([128, CH * SPG * BL], f32, tag="big")
            for c in range(CH):
                for half in range(2):
                    nc.tensor.matmul(
                        out=gbig[:, c * 256:(c + 1) * 256],
                        lhsT=keysT[:, half, c * 128:(c + 1) * 128],
                        rhs=encT[:, half, g * SPG * BL:(g + 1) * SPG * BL],
                        start=(half == 0), stop=(half == 1))
            eks = scratch.tile([128, CH, SPG, BL], f32, tag="ekscr")
            nc.vector.tensor_tensor(
                out=eks[:],
                in0=gbig[:].rearrange("p (c s b) -> p c s b", c=CH, s=SPG),
                in1=mdiag[:].unsqueeze(1).unsqueeze(1).broadcast_to(
                    [128, CH, SPG, BL]),
                op=MULT)
            red = scratch.tile([128, CH, SPG], f32, tag="ekred")
            nc.vector.tensor_reduce(
                out=red[:], in_=eks[:], axis=mybir.AxisListType.X, op=ADD)
            nc.vector.tensor_tensor(
                out=ekm[:, :, g * SPG:(g + 1) * SPG],
                in0=red[:],
                in1=bias_sb[:, g * SPG:(g + 1) * SPG].unsqueeze(1).broadcast_to(
                    [128, CH, SPG]),
                op=ADD)

        # kV[p, c, d] = keys @ V (needs only keysT)
        for c in range(CH):
            kvp = ps_sm.tile([128, D], f32, tag="sm")
            for half in range(2):
                nc.tensor.matmul(out=kvp[:],
                                 lhsT=keysT[:, half, c * 128:(c + 1) * 128],
                                 rhs=v_sb[:, half, :],
                                 start=(half == 0), stop=(half == 1))
            nc.vector.tensor_copy(out=kv[:, c, :], in_=kvp[:])

        SIG = mybir.ActivationFunctionType.Sigmoid
        SQT = mybir.ActivationFunctionType.Sqrt
        RELU = mybir.ActivationFunctionType.Relu

        def scan_step(s):
            last = (s == NSTEPS - 1)
            pn = ps_big.tile([128, CH * D], f32, tag="big")
            gps = ps_sm.tile([128, CH * BL], f32, tag="sm")
            # bias adds: psum[, c,] += Sel@eW_s  (+= kV via identity)
            for c in range(CH):
                nc.tensor.matmul(out=pn[:, c * D:(c + 1) * D], lhsT=selkm[:],
                                 rhs=ew_all[:, s * D:(s + 1) * D],
                                 start=(c % 2 == 0), stop=False)
            for c in range(CH):
                nc.tensor.matmul(out=pn[:, c * D:(c + 1) * D], lhsT=ident[:],
                                 rhs=kv[:, c, :], start=False, stop=False)
            # main h@U and gate h.e (half-major: half0 runs off copy0)
            for half in range(2):
                for c in range(CH):
                    lhs = hT[:, half, c * 128:(c + 1) * 128]
                    nc.tensor.matmul(out=pn[:, c * D:(c + 1) * D], lhsT=lhs,
                                     rhs=u_sb[:, half, :], start=False,
                                     stop=(half == 1 and c % 2 == 1))
                    nc.tensor.matmul(out=gps[:, c * BL:(c + 1) * BL],
                                     lhsT=lhs,
                                     rhs=encT[:, half, s * BL:(s + 1) * BL],
                                     start=(c == 0 and half == 0),
                                     stop=(c == CH - 1 and half == 1))
            # h_tilde = relu(psum) on ACT first (overlaps the gate DVE work)
            nc.scalar.activation(h_tld[:].rearrange("p c d -> p (c d)"),
                                 pn[:], RELU)
            # gate: mask diag, reduce, +EK, sigmoid
            nc.vector.tensor_tensor(
                out=gm[:], in0=gps[:].rearrange("p (c b) -> p c b", b=BL),
                in1=mdiag[:].unsqueeze(1).broadcast_to([128, CH, BL]),
                op=MULT)
            nc.vector.tensor_reduce(out=gpre[:], in_=gm[:],
                                    axis=mybir.AxisListType.X, op=ADD)
            nc.vector.tensor_tensor(out=gp2[:], in0=gpre[:],
                                    in1=ekm[:, :, s], op=ADD)
            nc.scalar.activation(g_all[:], gp2[:], SIG)
            # preload sqrt table set (dummy ordered after sigmoid)
            nc.scalar.activation(dum2[:], g_all[:, 0:1], SQT)
            # h_tld *= g (free-dim broadcast); upd = h_tld + h_nat
            nc.vector.tensor_tensor(
                out=h_tld[:],
                in0=h_tld[:],
                in1=g_all[:].unsqueeze(2).broadcast_to([128, CH, D]),
                op=MULT)
            nc.vector.tensor_tensor(
                out=upd[:].rearrange("p c d -> p (c d)"),
                in0=h_tld[:].rearrange("p c d -> p (c d)"),
                in1=h_nat[:].rearrange("p c d -> p (c d)"), op=ADD)
            # ss[c] = sum(upd^2)
            for c in range(CH):
                nc.vector.scalar_tensor_tensor(
                    out=sqd[:, c, :], in0=upd[:, c, :], scalar=1.0,
                    in1=upd[:, c, :], op0=MULT, op1=MULT,
                    accum_out=ss[:, c:c + 1])
            nc.scalar.activation(ssq[:], ss[:], SQT, bias=epsv[:])
            # preload sigmoid set for the next step (dummy ordered after sqrt)
            if not last:
                nc.scalar.activation(dum1[:], ssq[:, 0:1], SIG)
            nc.vector.reciprocal(out=r_all[:], in_=ssq[:])
            if not last:
                for c in range(CH):
                    nc.vector.tensor_scalar_mul(out=h_nat[:, c, :],
                                                in0=upd[:, c, :],
                                                scalar1=r_all[:, c:c + 1])
                # refresh hT on PE
                for half in range(2):
                    pt = ps_t.tile([128, J], bf16, tag="t_ps")
                    for c in range(CH):
                        nc.tensor.transpose(
                            out=pt[:, c * 128:(c + 1) * 128],
                            in_=h_nat[:, c, half * 128:(half + 1) * 128],
                            identity=ident[:])
                    if half == 0:
                        nc.vector.tensor_copy(out=hT[:, half, :], in_=pt[:])
                    else:
                        nc.scalar.copy(out=hT[:, half, :], in_=pt[:])
            else:
                for c in range(CH):
                    nc.vector.tensor_scalar_mul(out=hf32[:, c, :],
                                                in0=upd[:, c, :],
                                                scalar1=r_all[:, c:c + 1])

        if not SKIP_GATHER:
            raws = {0: emit_gather(0)}
            emit_group_precompute(0, raws.pop(0))
            for g in range(GRP):
                if g + 1 < GRP:
                    raws[g + 1] = emit_gather(g + 1)
                    emit_group_precompute(g + 1, raws.pop(g + 1))
                for si in range(SPG):
                    s = g * SPG + si
                    if s < NSTEPS:
                        scan_step(s)
        else:
            nc.vector.memset(encT[:], 0.0)
            nc.vector.memset(ew_all[:], 0.0)
            nc.vector.memset(ekm[:], 0.0)
            for s in range(NSTEPS):
                scan_step(s)

        if NSTEPS == 0:
            nc.vector.memset(hf32[:], 0.0)
        nc.sync.dma_start(out=dbg_d.ap()[:, 0:1], in_=dum1[:])
        nc.sync.dma_start(out=dbg_d.ap()[:, 1:2], in_=dum2[:])
        # ---- output: y[b, k, d] <- hf32[(k%4)*32+b, k//4, d] -------------
        y_main = y_d.ap()[:, 0:28, :].rearrange("b (kh kl) d -> b kl kh d",
                                                kl=4)
        for klo in range(4):
            nc.sync.dma_start(out=y_main[:, klo, :, :],
                              in_=hf32[klo * 32:(klo + 1) * 32, 0:7, :])
        nc.sync.dma_start(out=y_d.ap()[:, 28, :],
                          in_=hf32[0:32, 7, :])
        nc.sync.dma_start(out=y_d.ap()[:, 29, :],
                          in_=hf32[32:64, 7, :])

        for p in reversed(ctxs):
            p.__exit__(None, None, None)

    nc.compile()
    return nc


def _host_prep(prgrph, prgrph_mask, keys, embedding_matrix, U, V, W):
    """Build per-core input maps."""
    prg = np.asarray(prgrph).astype(np.int64)
    msk = np.asarray(prgrph_mask).astype(bool)
    keys = np.asarray(keys, dtype=np.float32)
    embm = np.asarray(embedding_matrix, dtype=np.float32)
    U = np.asarray(U, dtype=np.float32)
    V = np.asarray(V, dtype=np.float32)
    W = np.asarray(W, dtype=np.float32)

    emb_bf = embm.astype(_bf16)

    def halves(m):      # [256, 256] -> [128, 2, 256] bf16
        return np.ascontiguousarray(
            m.reshape(2, 128, D).swapaxes(0, 1).astype(_bf16))

    u_h, v_h, w_h = halves(U), halves(V), halves(W)

    ident = np.eye(128, dtype=_bf16)
    selsum = np.zeros((128, BL), dtype=_bf16)
    p_ar = np.arange(128)
    selsum[p_ar, p_ar % 32] = 1
    selkm = np.ascontiguousarray(selsum.T)
    mdiag = selsum.astype(np.float32)

    # token index layout: flat slot i=q*128+p, p=(l%4)*32+b, q=g*32+s_in*4+l//4
    tok = np.where(msk, prg, VOCAB).astype(np.int64)   # [B, S, L]
    sent_ok = msk.any(-1)                              # [B, S]

    in_maps = []
    for m in range(NCORES):
        b0 = m * BL
        t = tok[b0:b0 + BL]                            # [32, 64, 16]
        # quad dedup: one table row = the 4 l_hi embeddings of (b, s, l_lo)
        # quads[b, s, l_lo] = (t[b,s,l_lo], t[b,s,4+l_lo], t[b,s,8+l_lo], t[b,s,12+l_lo])
        quads = t.reshape(BL, S, 4, 4).transpose(0, 1, 3, 2)   # [b, s, l_lo, l_hi]
        qflat = np.ascontiguousarray(quads.reshape(-1, 4))
        uniq, inv = np.unique(qflat, axis=0, return_inverse=True)
        n_u = len(uniq)
        assert n_u <= 8192, f"unique quad overflow: {n_u}"
        emb_core = np.zeros((8192, 4, D), dtype=_bf16)
        safe = np.minimum(uniq, VOCAB)                  # VOCAB -> zero row
        ext = np.vstack([emb_bf, np.zeros((1, D), _bf16)])
        emb_core[:n_u] = ext[safe]
        emb_core = emb_core.reshape(8192, 4 * D)
        inv = inv.reshape(BL, S, 4)                     # [b, s, l_lo]
        # flat slot i = q*128 + p, p = l_lo*32 + b, q = s_in (per group)
        idx = np.zeros((128, 64), dtype=np.int16)       # [p, g*8+s_in]
        s_idx = np.arange(S)
        g_ar, si_ar = s_idx // SPG, s_idx % SPG
        for llo in range(4):
            p = llo * 32 + np.arange(BL)
            q = g_ar * 8 + si_ar
            idx[p[:, None], q[None, :]] = inv[:, :, llo].astype(np.int16)
        # wrap flat order i=q*128+p into [16, n/16] gather layout per group
        cols = []
        for g in range(GRP):
            flat = idx[:, g * 8:(g + 1) * 8].T.reshape(-1)   # i = s_in*128+p
            cols.append(flat.reshape(64, 16).T)
        idx16 = np.ascontiguousarray(np.tile(np.concatenate(cols, axis=1), (8, 1)))
        kT = np.zeros((D, J), dtype=_bf16)
        kloc = np.transpose(keys[b0:b0 + BL], (2, 1, 0))   # [D, K, BL]
        kT[:, :K * BL] = kloc.reshape(D, K * BL)[:, :]
        # j = k*32 + b -> reshape (K, BL) row-major matches k*32+b
        keysT_h = np.ascontiguousarray(kT.reshape(2, 128, J).swapaxes(0, 1))
        bias = np.zeros((128, S), dtype=np.float32)
        ok = sent_ok[b0:b0 + BL]                       # [32, 64]
        bias[:, :] = np.where(ok, 0.0, NEG)[np.arange(128) % 32, :]
        in_maps.append({
            "emb": emb_core, "idx": idx16, "keysT": keysT_h,
            "u": u_h, "v": v_h, "w": w_h, "bias": bias,
            "selsum": selsum, "selkm": selkm, "mdiag": mdiag,
            "ident": ident,
        })
    return in_maps


def kernel(**inputs):
    global LAST_EXEC_NS
    from concourse.bass_utils import run_bass_kernel_spmd

    trace = os.environ.get("BASS_KERNEL_TRACE", "0") == "1"
    if trace:
        try:
            import sys, types, contextlib

            if "antenv.axon_hooks" not in sys.modules:
                mod = types.ModuleType("antenv.axon_hooks")
                _h = [None]
                mod.set_axon_ntff_profile_hook = lambda h: _h.__setitem__(0, h)
                mod.get_axon_ntff_profile_hook = lambda: _h[0]
                sys.modules["antenv.axon_hooks"] = mod
                import antenv
                antenv.axon_hooks = mod
                from trn_agent_boot.trn_boot import _ntff_profile_via_ctypes
                mod.set_axon_ntff_profile_hook(
                    _ntff_profile_via_ctypes("/opt/axon/libaxon_pjrt.so"))
        except Exception as e:
            print("trace hook unavailable:", e)
            trace = False

    nc = _build_nc()
    in_maps = _host_prep(**inputs)
    res = run_bass_kernel_spmd(nc, in_maps, list(range(NCORES)), trace=trace)
    if trace:
        LAST_EXEC_NS = res.exec_time_ns
    out = np.concatenate([res.results[m]["y"] for m in range(NCORES)], axis=0)
    return out.astype(np.float32)

